# revision 10
# baseline (speedup 1.0000x reference)
"""Biaffine kernel for Trainium2, 8-core SPMD — OUT-sharded (v2).

logits[b,x,y,o] = sum_ij in1[b,x,i] * w1[i,o,j] * in2[b,y,j]
               + termA[b,x,o] + termB[b,y,o] + bias[o]

Sharding: core c owns the o-slice [14c, 14c+14) and computes ALL (b, x, y)
for it.  w1's o-slice (7.3 MB bf16) is SBUF-RESIDENT — no weight streaming
during the main loop (the previous x-sharded kernel streamed the full
58.7 MB w1 through every core, putting DMA on the tensor-engine ridge:
~360us DMA vs ~380us PE per core; this design needs only ~35 MB total
DMA per core, all overlappable).

Per (b, o):
  phase 1: temp[j, x] = sum_i w1[i,o,j] * in1[b,x,i]
           stationary = w1 128x128 tile (resident), moving = in1T [128, 512]
           -> 16 MMs of N=512 (ldweights ~107ns hides under the 213ns
           moving stream via the PE pull-ahead window), ACT drains
           PSUM->SBUF bf16.
  phase 2: out[x, y] = sum_jb temp[jb, x-tile]^T @ in2T[jb, y]
           -> 16 MMs of N=512; one fused DVE op drains PSUM AND applies
           the ENTIRE affine: out = (PSUM + termA[x,o]) + TBA[o]
           (scalar_tensor_tensor with per-partition scalar termA).
           TBA[o] = (termB[b,:,o]+bias[o]) row broadcast across partitions,
           built by ONE selector matmul per (b,o) emitted just-in-time one
           per o-iteration (termB rows themselves are precomputed in prep)
           — instead of one selector per output tile like the old kernel.
temp double-buffered: phase 1 of o+1 overlaps phase 2 of o.
Inputs are passed pre-cast to bf16 (everything consumes bf16; halves
input DMA) and in1T/in2T are built by xbar DMA transposes (k-major:
dst[p,k,x] = src[x, k*128+p], device-verified) with no PE/DVE involvement.
Output is written bf16 ([b, o, x, y]; +0.0008 rel err vs the 2e-2 gate)
and upcast/transposed to [b, x, y, o] fp32 on the host.

Measured (sim = concourse TimelineSim; HW = repeat-delta wall clock on the
axon-tunneled device, donation-based bench.py):
  old x-sharded kernel: sim 474us, HW main-loop ~557us/rep (harness: 739us)
  this kernel:          sim 440us, HW main-loop ~449us/rep
PE occupancy in sim is 90%+ (404us busy, ~97% of it required matmuls).
PSUM banks 4(ph1)+2(ph2)+1(selector), measured optimum: ph1 rotation depth
is the sensitive knob; ph2=2 suffices because the fused DVE drains keep
pace.  Rel err 0.0038 vs the 2e-2 gate.
"""

import numpy as np

B, S, IN, OUT = 4, 512, 512, 112
N_CORES = 8
P = 128
OC = OUT // N_CORES  # 14 o's per core


def split_sync_waits(nc, max_waits=1):
    """Hoist overflow semaphore waits onto NoOps (walrus rejects
    instructions with too many sync waits)."""
    import concourse.mybir as mybir

    n_split = 0
    for f in nc.m.functions:
        for bb in f.blocks:
            new_insts = []
            for inst in bb.instructions:
                si = inst.sync_info
                if si is not None and si.on_wait and len(si.on_wait) > max_waits:
                    waits = list(si.on_wait)
                    overflow, keep = waits[:-max_waits], waits[-max_waits:]
                    for k in range(0, len(overflow), max_waits):
                        chunk = overflow[k:k + max_waits]
                        nop = mybir.InstNoOp(
                            name=f"{inst.name}_wsplit{k}",
                            opcode="NoOp",
                            engine=inst.engine,
                            sync_info=mybir.SyncInfo(on_wait=chunk, on_update=[]),
                        )
                        new_insts.append(nop)
                        n_split += 1
                    si.on_wait = keep
                new_insts.append(inst)
            bb.instructions[:] = new_insts
    return n_split


def build_nc(S_=S, IN_=IN, OC_=OC, ps1_bufs=4, ps2_bufs=2, psb_bufs=1,
             temp_bufs=2, out_bufs=4, split_waits=True, repeat=1, only_phase=0,
             out_bf16=True, dma_tr=True):
    import concourse.bass as bass
    import concourse.mybir as mybir
    import concourse.tile as tile
    from concourse.masks import make_identity

    f32 = mybir.dt.float32
    bf16 = mybir.dt.bfloat16
    odt = bf16 if out_bf16 else f32

    KI = IN_ // P   # 128-blocks of the i/j contraction dims (4)
    XB = S_ // P    # x 128-blocks (4)

    nc = bass.Bass()
    in1 = nc.dram_tensor("in1", [B, S_, IN_], bf16, kind="ExternalInput")
    in2 = nc.dram_tensor("in2", [B, S_, IN_], bf16, kind="ExternalInput")
    w1s = nc.dram_tensor("w1s", [IN_, OC_, IN_], bf16, kind="ExternalInput")
    w2s = nc.dram_tensor("w2s", [2 * IN_ + 1, OC_], f32, kind="ExternalInput")
    outp = nc.dram_tensor("outp", [B, OC_, S_, S_], odt, kind="ExternalOutput")

    with tile.TileContext(nc) as tc:
        with tc.tile_pool(name="persist", bufs=1) as pers:
            in1T = pers.tile([P, B, KI, S_], bf16, name="in1T")
            in2T = pers.tile([P, B, KI, S_], bf16, name="in2T")
            w1sb = pers.tile([P, KI, OC_, IN_], bf16, name="w1sb")
            wAsb = pers.tile([P, KI, OC_], bf16, name="wAsb")
            wBsb = pers.tile([P, KI, OC_], bf16, name="wBsb")
            biasc = pers.tile([OC_, 1], f32, name="biasc")
            termA = pers.tile([P, B, XB, OC_], f32, name="termA")
            tbb_all = pers.tile([OC_, B, S_], bf16, name="tbb_all")
            TBA = pers.tile([P, 2, OC_, S_], bf16, name="TBA")  # per b-parity
            ident = pers.tile([P, P], f32, name="ident")
            identw = pers.tile([P, P], bf16, name="identw")

            # ---------------- prep: loads + transposes + termA ----------------
            with tc.tile_pool(name="prep", bufs=2) as prep, \
                 tc.tile_pool(name="prep_ps", bufs=2, space="PSUM") as prep_ps:
                make_identity(nc, ident)
                nc.vector.tensor_copy(identw, ident)

                def transpose_into(dst, src_dram):
                    # src_dram: [S_, IN_] bf16 -> dst [P, KI, S_] bf16 (= src^T)
                    if dma_tr:
                        # xbar DMA transpose straight from DRAM: no PE/DVE
                        # involvement.  Layout is k-major (dst[p,k,x] =
                        # src[x, k*128+p]) — verified by device probe.
                        nc.sync.dma_start(dst[:, :, :], src_dram[:, :],
                                          transpose=True)
                        return
                    # staged in XB chunks so the first transpose starts after
                    # ~1/4 of the load; bf16 PE transposes run 1 cycle/row and
                    # the bf16 PSUM->SBUF copies get the 2x DVE mode
                    st = prep.tile([P, XB, IN_], bf16, name="stage", tag="stage")
                    for a in range(XB):
                        nc.sync.dma_start(
                            st[:, a, :],
                            src_dram[a * P:(a + 1) * P, :].rearrange(
                                "(a p) i -> p (a i)", p=P))
                    for a in range(XB):
                        for ib in range(KI):
                            pt = prep_ps.tile([P, P], bf16, name="pt", tag="pt")
                            nc.tensor.transpose(pt, st[:, a, ib * P:(ib + 1) * P],
                                                identw)
                            nc.vector.tensor_copy(dst[:, ib, a * P:(a + 1) * P], pt)

                # input stages go FIRST so the transpose pipeline starts
                # immediately; the (big) resident-w1 load queues behind them
                # and finishes well before the first main-loop matmul needs it.
                transpose_into(in1T[:, 0], in1[0])
                wABf = prep.tile([P, 2, KI, OC_], f32, name="wABf", tag="wABf")
                nc.sync.dma_start(
                    wABf[:, 0], w2s[0:IN_, :].rearrange("(a p) o -> p a o", p=P))
                nc.sync.dma_start(
                    wABf[:, 1], w2s[IN_:2 * IN_, :].rearrange("(a p) o -> p a o", p=P))
                nc.vector.tensor_copy(wAsb, wABf[:, 0])
                nc.vector.tensor_copy(wBsb, wABf[:, 1])
                with nc.allow_non_contiguous_dma(reason="56B one-time bias load"):
                    nc.sync.dma_start(
                        biasc, w2s[2 * IN_:2 * IN_ + 1, :].rearrange("a o -> o a"))
                transpose_into(in2T[:, 0], in2[0])
                for b in range(1, B):
                    transpose_into(in1T[:, b], in1[b])
                    transpose_into(in2T[:, b], in2[b])
                # w1 load last: splitting it into an urgent first o-chunk or
                # hoisting it ahead of the b1-3 input loads measured WORSE
                # (444.8us vs 440.0us) — the main loop's early iterations do
                # not actually wait on it.
                for ib in range(KI):
                    nc.sync.dma_start(w1sb[:, ib, :, :],
                                      w1s[ib * P:(ib + 1) * P, :, :])

                # termA[x, o] = sum_i in1[b,x,i] * wA[i,o]
                for b in range(B):
                    for xb in range(XB):
                        psA = prep_ps.tile([P, OC_], f32, name="psA", tag="psA")
                        for ib in range(KI):
                            nc.tensor.matmul(
                                psA, in1T[:, b, ib, xb * P:(xb + 1) * P],
                                wAsb[:, ib, :],
                                start=(ib == 0), stop=(ib == KI - 1))
                        nc.vector.tensor_copy(termA[:, b, xb, :], psA)

                # tbb_all[o, b, y] = termB[b, y, o] + bias[o]
                for b in range(B):
                    psTB = prep_ps.tile([OC_, S_], f32, name="psTB", tag="psTB")
                    for jb in range(KI):
                        nc.tensor.matmul(psTB, wBsb[:, jb, :], in1T[:, b, jb, :],
                                         start=(jb == 0), stop=(jb == KI - 1))
                    nc.vector.tensor_scalar_add(tbb_all[:, b, :], psTB, biasc)

            # ---------------- main: per (b, o) two-phase pipeline ----------------
            with tc.tile_pool(name="tempp", bufs=temp_bufs) as tempp, \
                 tc.tile_pool(name="outsb", bufs=out_bufs) as outsb, \
                 tc.tile_pool(name="ps1", bufs=ps1_bufs, space="PSUM") as ps1p, \
                 tc.tile_pool(name="ps2", bufs=ps2_bufs, space="PSUM") as ps2p, \
                 tc.tile_pool(name="psb", bufs=psb_bufs, space="PSUM") as psbp:
                def tba_selector(b, slot, ol):
                    # TBA[slot, ol, :] = tbb_all[ol, b, :] bcast over partitions
                    psb = psbp.tile([P, S_], f32, name="psb", tag="psb")
                    nc.tensor.matmul(
                        psb, identw[0:OC_, ol:ol + 1].to_broadcast((OC_, P)),
                        tbb_all[:, b, :], start=True, stop=True)
                    nc.scalar.activation(TBA[:, slot, ol, :], psb,
                                         mybir.ActivationFunctionType.Copy)

                bseq = [bb for _ in range(repeat) for bb in range(B)]
                for bi, b in enumerate(bseq):
                    par = bi % 2
                    for ol in range(OC_):
                        # one selector per iteration, just-in-time (termB rows
                        # all precomputed in prep; psb double-buffered so even
                        # scheduler-bunched selectors don't stall the PE)
                        tba_selector(b, par, ol)
                        # phase 1: temp[j, x] for this (b, o)
                        # (pairing jb's into 2-bank psum tiles with one wide
                        # ACT drain measured neutral-to-worse; keep singles)
                        temp = tempp.tile([P, KI, S_], bf16, name="temp", tag="temp")
                        for jb in range(KI) if only_phase in (0, 1) else []:
                            ps1 = ps1p.tile([P, S_], f32, name="ps1", tag="ps1")
                            for ib in range(KI):
                                nc.tensor.matmul(
                                    ps1, w1sb[:, ib, ol, jb * P:(jb + 1) * P],
                                    in1T[:, b, ib, :],
                                    start=(ib == 0), stop=(ib == KI - 1))
                            nc.scalar.activation(
                                temp[:, jb, :], ps1,
                                mybir.ActivationFunctionType.Copy)
                        # phase 2: out[x, y] + affine
                        for xb in range(XB) if only_phase in (0, 2) else []:
                            ps2 = ps2p.tile([P, S_], f32, name="ps2", tag="ps2")
                            for jb in range(KI):
                                nc.tensor.matmul(
                                    ps2, temp[:, jb, xb * P:(xb + 1) * P],
                                    in2T[:, b, jb, :],
                                    start=(jb == 0), stop=(jb == KI - 1))
                            ot = outsb.tile([P, S_], odt, name="ot", tag="ot")
                            nc.vector.scalar_tensor_tensor(
                                ot, ps2, termA[:, b, xb, ol:ol + 1],
                                TBA[:, par, ol, :],
                                mybir.AluOpType.add, mybir.AluOpType.add)
                            nc.sync.dma_start(
                                outp[b, ol, xb * P:(xb + 1) * P, :], ot)

    if split_waits:
        # max_waits=1 is a HARD walrus limit: 2-deep waits fail codegen
        # (setupSyncWait, CoreV2GenImpl.cpp:176) — verified 2026-08.
        split_sync_waits(nc)
    return nc


_CACHE = {}


def _get_nc(**kw):
    key = tuple(sorted(kw.items()))
    if key not in _CACHE:
        _CACHE[key] = build_nc(**kw)
    return _CACHE[key]


TRACE = False
OUT_BF16 = True
LAST_RESULT = None


def kernel(input1, input2, w1, w2, seq_len=None, **_ignored):
    global LAST_RESULT
    from concourse.bass_utils import run_bass_kernel_spmd
    import ml_dtypes

    input1 = np.asarray(input1, dtype=np.float32)
    input2 = np.asarray(input2, dtype=np.float32)
    w1 = np.asarray(w1, dtype=np.float32)
    w2 = np.asarray(w2, dtype=np.float32)

    nc = _get_nc(out_bf16=OUT_BF16)
    w1b = w1.astype(ml_dtypes.bfloat16)

    in1b = input1.astype(ml_dtypes.bfloat16)
    in2b = input2.astype(ml_dtypes.bfloat16)
    in_maps = []
    for c in range(N_CORES):
        o0 = c * OC
        in_maps.append({
            "in1": in1b,
            "in2": in2b,
            "w1s": np.ascontiguousarray(w1b[:, o0:o0 + OC, :]),
            "w2s": np.ascontiguousarray(w2[:, o0:o0 + OC]),
        })
    res = run_bass_kernel_spmd(nc, in_maps, core_ids=list(range(N_CORES)),
                               trace=TRACE)
    LAST_RESULT = res

    full = np.empty((B, S, S, OUT), dtype=np.float32)
    for c in range(N_CORES):
        o0 = c * OC
        # device layout [b, o, x, y] -> [b, x, y, o]
        full[:, :, :, o0:o0 + OC] = np.asarray(
            res.results[c]["outp"], dtype=np.float32).transpose(0, 2, 3, 1)
    return full
